# revision 2
# baseline (speedup 1.0000x reference)
"""Trainium2 Bass kernel for CosmicMultiHeadAttention (block-local flash attention).

Sharding: the 8192 tokens (B=2 x S=4096) are split into 8 shards of 1024
tokens (batch-major). Attention is block-local with 128-token blocks, so
1024-token shards (8 blocks each) have zero cross-shard dependencies: every
core runs the full layer (QKV proj + RoPE + block attention + out proj) for
its own tokens. No collectives.

Single-pass-weights structure (vs the per-half baseline):
  Phase A: x kept fully resident as xT [E, 1024] (64KB/partition); wv, wk,
    wq each streamed from HBM exactly once; V, K, Q projections computed
    for all 1024 tokens (psum N=512 halves inner). RoPE fused into psum
    eviction (ACT copy + partition-shift DMA + in-place DVE muls).
  Phase B: attention per (block, kv-head), 4 grouped q-heads (N=512).
    sT = kT.T @ qT; exp on ACT; l = ones-matmul; the softmax reciprocal is
    computed as exp(-ln(l)) on ACT (Ln on the [1,512] row, broadcast by a
    K=1 matmul, Exp(scale=-1) on the [128,512] eviction) - the DVE
    RECIPROCAL instruction is 8 cyc/elem on one lane (3.3us!) and was the
    baseline's attention bottleneck. One-iteration software pipeline keeps
    ACT dense. oT reuses xT's SBUF space (LIFO pool scoping).
  Phase C: out projection, wo streamed once in 1MB chunks, all 8 psum
    banks accumulate the 8 token-chunks of one 512-col tile; evictions
    overlap the next tile's accumulation.
"""

import sys

if '/opt/trn_rl_repo' not in sys.path:
    sys.path.insert(0, '/opt/trn_rl_repo')

import numpy as np
import ml_dtypes

import concourse.bass as bass
import concourse.tile as tile
from concourse import mybir
from concourse.bass_utils import run_bass_kernel_spmd

BF16 = mybir.dt.bfloat16
F32 = mybir.dt.float32
NPBF16 = ml_dtypes.bfloat16
AF = mybir.ActivationFunctionType

B, S, E = 2, 4096, 4096
HQ, HKV, D = 32, 8, 128
BS = 128
ROPE_BASE = 10000.0
NCORES = 8
TOK = (B * S) // NCORES          # 1024 tokens per core
KO = E // 128                    # 32 k-tiles over E
MQ = (HQ * D) // 128             # 32 q head-tiles
MK = (HKV * D) // 128            # 8 k head-tiles
G = HQ // HKV                    # 4 q heads per kv head
NBLK = TOK // BS                 # 8 blocks per core
NE = E // 512                    # 8 out-proj column tiles
SCALE = 1.0 / float(np.sqrt(D))


# ---------------------------------------------------------------------------
# The walrus build in this image rejects instructions carrying more than one
# "sem-ge" sync wait ("Too many sync wait commands"; Drain/CTRL accepts
# none). Tile's scheduler freely attaches several. Post-pass: keep at most
# one ge-wait per instruction (none on Drain) and move the excess onto
# EventSemaphore carrier instructions inserted immediately before, on the
# same engine - program order preserves the blocking semantics exactly.
# ---------------------------------------------------------------------------
def _split_excess_waits(nc):
    import bass_rust
    ctr = 0
    for f in nc.m.functions:
        for bb in f.blocks:
            out_list = []
            for inst in bb.instructions:
                si = inst.sync_info
                all_waits = list(si.on_wait) if si and si.on_wait else []
                ge = [w for w in all_waits if 'ge' in w.wait_mode]
                eq = [w for w in all_waits if 'ge' not in w.wait_mode]
                keep_n = 0 if type(inst).__name__ == 'InstDrain' else 1
                if len(ge) > keep_n:
                    extra, keep = ge[:-keep_n] if keep_n else ge, \
                        ge[-keep_n:] if keep_n else []
                    for w in extra:
                        ctr += 1
                        es = mybir.InstEventSemaphore(
                            name=f'waitsplit_{ctr}', engine=inst.engine,
                            ins=[], outs=[],
                            sync_info=bass_rust.SyncInfo(
                                on_wait=[w], on_update=[]))
                        out_list.append(es)
                    si.on_wait = eq + keep
                out_list.append(inst)
            bb.instructions[:] = out_list
    return nc


def _build(flags):
    use_bias, use_mask = flags
    nc = bass.Bass()

    xT = nc.dram_tensor("xT", [KO, 128, TOK], BF16, kind="ExternalInput")
    wq_t = nc.dram_tensor("wq_t", [MQ, 128, KO, 128], BF16, kind="ExternalInput")
    wk_t = nc.dram_tensor("wk_t", [MK, 128, KO, 128], BF16, kind="ExternalInput")
    wv_t = nc.dram_tensor("wv_t", [2, 128, KO, 512], BF16, kind="ExternalInput")
    wo_t = nc.dram_tensor("wo_t", [NE, 128, MQ, 512], BF16, kind="ExternalInput")
    cos_t = nc.dram_tensor("cos_t", [128, TOK], BF16, kind="ExternalInput")
    sin_t = nc.dram_tensor("sin_t", [128, TOK], BF16, kind="ExternalInput")
    if use_mask:
        mb_t = nc.dram_tensor("mb_t", [NBLK, 128], F32, kind="ExternalInput")
    if use_bias:
        bq_t = nc.dram_tensor("bq_t", [MQ, 128], F32, kind="ExternalInput")
        bk_t = nc.dram_tensor("bk_t", [MK, 128], F32, kind="ExternalInput")
        bv_t = nc.dram_tensor("bv_t", [2, 512], BF16, kind="ExternalInput")
        bo_t = nc.dram_tensor("bo_t", [NE, 512], BF16, kind="ExternalInput")
    out = nc.dram_tensor("out", [TOK, E], F32, kind="ExternalOutput")

    with tile.TileContext(nc) as tc:
        with tc.tile_pool(name="const", bufs=1) as cpool:
            cos_sb = cpool.tile([128, TOK], BF16, tag="cos")
            nc.sync.dma_start(cos_sb[:], cos_t[:, :])
            sin_sb = cpool.tile([128, TOK], BF16, tag="sin")
            nc.sync.dma_start(sin_sb[:], sin_t[:, :])
            ones_col = cpool.tile([128, 1], BF16, tag="ones_col")
            nc.vector.memset(ones_col[:], 1.0)
            ones_row = cpool.tile([1, 128], BF16, tag="ones_row")
            nc.vector.memset(ones_row[:], 1.0)
            ones_row_f = cpool.tile([1, 128], F32, tag="ones_row_f")
            nc.vector.memset(ones_row_f[:], 1.0)
            if use_mask:
                mb_sb = cpool.tile([128, NBLK], F32, tag="mb")
                nc.sync.dma_start(mb_sb[:], mb_t.rearrange("b p -> p b"))
            if use_bias:
                bq_sb = cpool.tile([128, MQ], F32, tag="bq")
                nc.sync.dma_start(bq_sb[:], bq_t.rearrange("m p -> p m"))
                bk_sb = cpool.tile([128, MK], F32, tag="bk")
                nc.sync.dma_start(bk_sb[:], bk_t.rearrange("m p -> p m"))
                bv_sb = cpool.tile([1, 2, 512], BF16, tag="bv")
                nc.sync.dma_start(bv_sb[:], bv_t[None, :, :])
                bo_sb = cpool.tile([1, NE, 512], BF16, tag="bo")
                nc.sync.dma_start(bo_sb[:], bo_t[None, :, :])

            with tc.tile_pool(name="persist", bufs=1) as ppool:
                qT = ppool.tile([128, MQ, TOK], BF16, tag="qT")
                kT = ppool.tile([128, MK, TOK], BF16, tag="kT")
                v_sb = ppool.tile([128, NBLK, HKV * D], BF16, tag="v")

                # ============ Phase A: projections, weights one pass ======
                with (
                    tc.tile_pool(name="xt", bufs=1) as xpool,
                    tc.tile_pool(name="wstream", bufs=2) as wpool,
                    tc.tile_pool(name="rope", bufs=2) as rpool,
                    tc.tile_pool(name="psA", bufs=8, space="PSUM") as psA,
                ):
                    xt = xpool.tile([128, KO, TOK], BF16, tag="xt")
                    for ko in range(KO):
                        nc.sync.dma_start(xt[:, ko, :], xT[ko])

                    # ---- V projection: psum rows = tokens ----
                    for half in range(2):
                        for n in range(2):
                            pss = [psA.tile([128, 512], F32, tag="ps",
                                            name=f"psv{half}{n}_{i}")
                                   for i in range(4)]
                            for kc in range(4):
                                wvs = wpool.tile([128, 8, 512], BF16, tag="wv")
                                nc.sync.dma_start(
                                    wvs[:], wv_t[n, :, kc * 8:(kc + 1) * 8, :])
                                for k8 in range(8):
                                    ko = kc * 8 + k8
                                    for mt in range(4):
                                        c0 = half * 512 + mt * 128
                                        nc.tensor.matmul(
                                            pss[mt][:],
                                            xt[:, ko, c0:c0 + 128],
                                            wvs[:, k8, :],
                                            start=(ko == 0),
                                            stop=(ko == KO - 1 and not use_bias))
                            if use_bias:
                                for mt in range(4):
                                    nc.tensor.matmul(pss[mt][:], ones_row[:],
                                                     bv_sb[:, n, :],
                                                     start=False, stop=True)
                            for mt in range(4):
                                nc.scalar.copy(
                                    v_sb[:, half * 4 + mt,
                                         n * 512:(n + 1) * 512], pss[mt][:])

                    # ---- K/Q projections with fused RoPE eviction ----
                    def rope_evict(ps, dst_ap, half, bias_sb, m):
                        qa = rpool.tile([128, 512], BF16, tag="qa")
                        if bias_sb is not None:
                            nc.scalar.add(qa[:], ps[:], bias_sb[:, m:m + 1])
                        else:
                            nc.scalar.copy(qa[:], ps[:])
                        qsh = rpool.tile([128, 512], BF16, tag="qsh")
                        nc.sync.dma_start(qsh[0:64, :], qa[64:128, :])
                        nc.sync.dma_start(qsh[64:128, :], qa[0:64, :])
                        ts = slice(half * 512, (half + 1) * 512)
                        nc.vector.tensor_mul(qa[:], qa[:], cos_sb[:, ts])
                        nc.vector.tensor_mul(qsh[:], qsh[:], sin_sb[:, ts])
                        nc.vector.tensor_add(dst_ap, qa[:], qsh[:])

                    def qk_proj(nm, w_dram, dst, bias_sb):
                        for m in range(nm):
                            wsb = wpool.tile([128, KO, 128], BF16, tag="wqk")
                            nc.sync.dma_start(wsb[:], w_dram[m])
                            for half in range(2):
                                ps = psA.tile([128, 512], F32, tag="ps")
                                for ko in range(KO):
                                    nc.tensor.matmul(
                                        ps[:], wsb[:, ko, :],
                                        xt[:, ko,
                                           half * 512:(half + 1) * 512],
                                        start=(ko == 0), stop=(ko == KO - 1))
                                rope_evict(
                                    ps,
                                    dst[:, m, half * 512:(half + 1) * 512],
                                    half, bias_sb, m)

                    qk_proj(MK, wk_t, kT, bk_sb if use_bias else None)
                    qk_proj(MQ, wq_t, qT, bq_sb if use_bias else None)

                # ============ Phase B: block-local attention ==============
                # oT reuses the SBUF space xt just freed.
                with tc.tile_pool(name="oT", bufs=1) as opool:
                    oT = opool.tile([128, MQ, TOK], BF16, tag="oT")
                    with (
                        tc.tile_pool(name="attn", bufs=3) as apool,
                        tc.tile_pool(name="lr", bufs=2) as lrpool,
                        tc.tile_pool(name="bcb", bufs=2) as bpool,
                        tc.tile_pool(name="ps_s", bufs=2, space="PSUM") as ps_s,
                        tc.tile_pool(name="ps_l", bufs=2, space="PSUM") as ps_l,
                        tc.tile_pool(name="ps_bc", bufs=2, space="PSUM") as ps_bc,
                        tc.tile_pool(name="ps_pv", bufs=2, space="PSUM") as ps_pv,
                    ):
                        pending = None

                        def finish(p):
                            l_ps, pv_ps, dst = p
                            # 1/l as exp(-ln(l)): Ln on the [1,512] row,
                            # K=1 matmul broadcast, Exp(-x) on eviction.
                            lr = lrpool.tile([1, 512], F32, tag="lr")
                            nc.scalar.activation(out=lr[:], in_=l_ps[:],
                                                 func=AF.Ln)
                            bc_ps = ps_bc.tile([128, 512], F32, tag="bc")
                            nc.tensor.matmul(bc_ps[:], ones_row_f[:], lr[:],
                                             start=True, stop=True)
                            bcb = bpool.tile([128, 512], F32, tag="bcb")
                            nc.scalar.activation(out=bcb[:], in_=bc_ps[:],
                                                 func=AF.Exp, scale=-1.0)
                            nc.vector.tensor_mul(dst, pv_ps[:], bcb[:])

                        for blk in range(NBLK):
                            tq = slice(blk * 128, (blk + 1) * 128)
                            for kvh in range(HKV):
                                s_ps = ps_s.tile([128, 512], F32, tag="s")
                                nc.tensor.matmul(
                                    s_ps[:], kT[:, kvh, tq],
                                    qT[:, kvh * G:(kvh + 1) * G, tq],
                                    start=True, stop=True)
                                wT = apool.tile([128, 512], BF16, tag="wT")
                                if use_mask:
                                    nc.scalar.activation(
                                        out=wT[:], in_=s_ps[:], func=AF.Exp,
                                        scale=SCALE,
                                        bias=mb_sb[:, blk:blk + 1])
                                else:
                                    nc.scalar.activation(
                                        out=wT[:], in_=s_ps[:], func=AF.Exp,
                                        scale=SCALE)
                                l_ps = ps_l.tile([1, 512], F32, tag="l")
                                nc.tensor.matmul(l_ps[:], ones_col[:], wT[:],
                                                 start=True, stop=True)
                                pv_ps = ps_pv.tile([128, 512], F32, tag="pv")
                                nc.tensor.matmul(
                                    pv_ps[:],
                                    v_sb[:, blk, kvh * 128:(kvh + 1) * 128],
                                    wT[:], start=True, stop=True)
                                if pending is not None:
                                    finish(pending)
                                pending = (l_ps, pv_ps,
                                           oT[:, kvh * G:(kvh + 1) * G, tq])
                        finish(pending)

                    # ============ Phase C: out projection ================
                    with (
                        tc.tile_pool(name="wo", bufs=3) as wopool,
                        tc.tile_pool(name="oe", bufs=4) as oepool,
                        tc.tile_pool(name="psC", bufs=8, space="PSUM") as psC,
                    ):
                        for n in range(NE):
                            pss = [psC.tile([128, 512], F32, tag="ps",
                                            name=f"pso{n}_{i}")
                                   for i in range(8)]
                            for kc in range(4):
                                wos = wopool.tile([128, 8, 512], BF16,
                                                  tag="wo")
                                nc.sync.dma_start(
                                    wos[:],
                                    wo_t[n, :, kc * 8:(kc + 1) * 8, :])
                                for grp in range(2):
                                    for h8 in range(8):
                                        hk = kc * 8 + h8
                                        for mt in range(4):
                                            i = grp * 4 + mt
                                            t0 = i * 128
                                            nc.tensor.matmul(
                                                pss[i][:],
                                                oT[:, hk, t0:t0 + 128],
                                                wos[:, h8, :],
                                                start=(hk == 0),
                                                stop=(hk == MQ - 1
                                                      and not use_bias))
                            if use_bias:
                                for i in range(8):
                                    nc.tensor.matmul(pss[i][:], ones_row[:],
                                                     bo_sb[:, n, :],
                                                     start=False, stop=True)
                            for i in range(8):
                                oe = oepool.tile([128, 512], F32, tag="oe")
                                nc.scalar.copy(oe[:], pss[i][:])
                                r0 = i * 128
                                nc.sync.dma_start(
                                    out[r0:r0 + 128, n * 512:(n + 1) * 512],
                                    oe[:])

    return _split_excess_waits(nc)


_NC_CACHE = {}


def _get_nc(flags):
    if flags not in _NC_CACHE:
        _NC_CACHE[flags] = _build(flags)
    return _NC_CACHE[flags]


def _prepare(x, wq, bq, wk, bk, wv, bv, wo, bo, mask):
    x = np.asarray(x, np.float32)
    wq = np.asarray(wq, np.float32)
    wk = np.asarray(wk, np.float32)
    wv = np.asarray(wv, np.float32)
    wo = np.asarray(wo, np.float32)
    bq = np.asarray(bq, np.float32)
    bk = np.asarray(bk, np.float32)
    bv = np.asarray(bv, np.float32)
    bo = np.asarray(bo, np.float32)
    mask = np.asarray(mask)

    use_bias = bool(bq.any() or bk.any() or bv.any() or bo.any())
    use_mask = not bool(mask.all())

    # weight layouts (shared across cores)
    wq_t = np.ascontiguousarray(
        wq.reshape(KO, 128, MQ, 128).transpose(2, 1, 0, 3)).astype(NPBF16)
    wk_t = np.ascontiguousarray(
        wk.reshape(KO, 128, MK, 128).transpose(2, 1, 0, 3)).astype(NPBF16)
    wv_t = np.ascontiguousarray(
        wv.reshape(KO, 128, 2, 512).transpose(2, 1, 0, 3)).astype(NPBF16)
    wo_t = np.ascontiguousarray(
        wo.reshape(MQ, 128, NE, 512).transpose(2, 1, 0, 3)).astype(NPBF16)

    # RoPE tables (positions are global sequence positions)
    inv = 1.0 / (ROPE_BASE ** (np.arange(0, D, 2, dtype=np.float32) / D))
    pos = np.arange(S, dtype=np.float32)
    ang = pos[:, None] * inv[None, :]                      # [S, 64]
    cos_full = np.concatenate([np.cos(ang), np.cos(ang)], -1).T  # [128, S]
    sin_half = np.sin(ang).T                               # [64, S]
    sin_eff = np.concatenate([-sin_half, sin_half], 0)     # [128, S]

    shards_per_b = NCORES // B                             # 4
    in_maps = []
    for c in range(NCORES):
        b = c // shards_per_b
        s0 = (c % shards_per_b) * TOK
        xs = x[b, s0:s0 + TOK]                             # [TOK, E]
        xTs = np.ascontiguousarray(xs.T).astype(NPBF16)    # [E, TOK]
        im = {
            "xT": xTs.reshape(KO, 128, TOK),
            "wq_t": wq_t, "wk_t": wk_t, "wv_t": wv_t, "wo_t": wo_t,
            "cos_t": np.ascontiguousarray(cos_full[:, s0:s0 + TOK]).astype(NPBF16),
            "sin_t": np.ascontiguousarray(sin_eff[:, s0:s0 + TOK]).astype(NPBF16),
        }
        if use_mask:
            mshard = mask[b, s0:s0 + TOK].reshape(NBLK, BS)
            im["mb_t"] = np.where(mshard, np.float32(0.0),
                                  np.float32(-80.0)).astype(np.float32)
        if use_bias:
            im["bq_t"] = bq.reshape(MQ, 128).copy()
            im["bk_t"] = bk.reshape(MK, 128).copy()
            im["bv_t"] = bv.reshape(2, 512).astype(NPBF16)
            im["bo_t"] = bo.reshape(NE, 512).astype(NPBF16)
        in_maps.append(im)

    return in_maps, (use_bias, use_mask)


def _assemble(results):
    shards_per_b = NCORES // B
    out = np.empty((B, S, E), np.float32)
    for c in range(NCORES):
        b = c // shards_per_b
        s0 = (c % shards_per_b) * TOK
        out[b, s0:s0 + TOK] = results[c]["out"]
    return out


def kernel(**inputs):
    in_maps, flags = _prepare(**inputs)
    nc = _get_nc(flags)
    res = run_bass_kernel_spmd(nc, in_maps, core_ids=list(range(NCORES)))
    return _assemble(res.results)


# revision 5
# speedup vs baseline: 1.1312x; 1.1312x over previous
"""Trainium2 Bass kernel for CosmicMultiHeadAttention (block-local flash attention).

Sharding: the 8192 tokens (B=2 x S=4096) are split into 8 shards of 1024
tokens (batch-major). Attention is block-local with 128-token blocks, so
1024-token shards (8 blocks each) have zero cross-shard dependencies: every
core runs the full layer (QKV proj + RoPE + block attention + out proj) for
its own tokens. No collectives.

Single-pass-weights structure, attention interleaved into the Q projection:
  Phase A: x kept fully resident as xT [E, 1024] (64KB/partition); wv, wk,
    wq each streamed from HBM exactly once. V proj first (psum rows =
    tokens), interleaved with the xT load; then K proj; then Q proj.
    RoPE fused into the q/k psum eviction.
  Attention rides inside the Q-proj loop: after each (m-tile, half) slot
    (6.8us of projection matmuls) one attention iteration (block, kv-head,
    4 grouped q-heads) is emitted in a 3-stage software pipeline, so its
    ACT/DVE work hides under projection matmuls and the PE stays warm.
    Softmax denominators: l = ones-matmul; rc = exp(-ln l) on ACT (bf16),
    broadcast over partitions by a bf16 K=1 matmul (an fp32 broadcast
    matmul runs LOW_HIGH = 2 passes = 2.3us - bf16 is 213ns); final
    normalize on DVE. The attention output overwrites the consumed q-head
    slices of qT in place (block-granular WAR, Tile-tracked), so no extra
    SBUF is needed and the out projection just reads qT.
  Phase C: out projection, wo streamed once in 1MB chunks, all 8 psum
    banks accumulate the 8 token-chunks of one 512-col tile; evictions
    overlap the next tile's accumulation.
"""

import sys

if '/opt/trn_rl_repo' not in sys.path:
    sys.path.insert(0, '/opt/trn_rl_repo')

import numpy as np
import ml_dtypes

import concourse.bass as bass
import concourse.tile as tile
from concourse import mybir
from concourse.bass_utils import run_bass_kernel_spmd

BF16 = mybir.dt.bfloat16
F32 = mybir.dt.float32
NPBF16 = ml_dtypes.bfloat16
AF = mybir.ActivationFunctionType

B, S, E = 2, 4096, 4096
HQ, HKV, D = 32, 8, 128
BS = 128
ROPE_BASE = 10000.0
NCORES = 8
TOK = (B * S) // NCORES          # 1024 tokens per core
KO = E // 128                    # 32 k-tiles over E
MQ = (HQ * D) // 128             # 32 q head-tiles
MK = (HKV * D) // 128            # 8 k head-tiles
G = HQ // HKV                    # 4 q heads per kv head
NBLK = TOK // BS                 # 8 blocks per core
NE = E // 512                    # 8 out-proj column tiles
SCALE = 1.0 / float(np.sqrt(D))


# ---------------------------------------------------------------------------
# The walrus build in this image rejects instructions carrying more than one
# "sem-ge" sync wait ("Too many sync wait commands"; Drain/CTRL accepts
# none). Tile's scheduler freely attaches several. Post-pass: keep at most
# one ge-wait per instruction (none on Drain) and move the excess onto
# EventSemaphore carrier instructions inserted immediately before, on the
# same engine - program order preserves the blocking semantics exactly.
# ---------------------------------------------------------------------------
def _split_excess_waits(nc):
    import bass_rust
    ctr = 0
    for f in nc.m.functions:
        for bb in f.blocks:
            out_list = []
            for inst in bb.instructions:
                si = inst.sync_info
                all_waits = list(si.on_wait) if si and si.on_wait else []
                ge = [w for w in all_waits if 'ge' in w.wait_mode]
                eq = [w for w in all_waits if 'ge' not in w.wait_mode]
                keep_n = 0 if type(inst).__name__ == 'InstDrain' else 1
                if len(ge) > keep_n:
                    extra, keep = ge[:-keep_n] if keep_n else ge, \
                        ge[-keep_n:] if keep_n else []
                    for w in extra:
                        ctr += 1
                        es = mybir.InstEventSemaphore(
                            name=f'waitsplit_{ctr}', engine=inst.engine,
                            ins=[], outs=[],
                            sync_info=bass_rust.SyncInfo(
                                on_wait=[w], on_update=[]))
                        out_list.append(es)
                    si.on_wait = eq + keep
                out_list.append(inst)
            bb.instructions[:] = out_list
    return nc


def _build(flags):
    use_bias, use_mask = flags
    nc = bass.Bass()

    xT = nc.dram_tensor("xT", [KO, 128, TOK], BF16, kind="ExternalInput")
    wq_t = nc.dram_tensor("wq_t", [MQ, 128, KO, 128], BF16, kind="ExternalInput")
    wk_t = nc.dram_tensor("wk_t", [MK, 128, KO, 128], BF16, kind="ExternalInput")
    wv_t = nc.dram_tensor("wv_t", [2, 128, KO, 512], BF16, kind="ExternalInput")
    wo_t = nc.dram_tensor("wo_t", [NE, 128, MQ, 512], BF16, kind="ExternalInput")
    cos_t = nc.dram_tensor("cos_t", [128, TOK], BF16, kind="ExternalInput")
    sin_t = nc.dram_tensor("sin_t", [128, TOK], BF16, kind="ExternalInput")
    if use_mask:
        mb_t = nc.dram_tensor("mb_t", [NBLK, 128], F32, kind="ExternalInput")
    if use_bias:
        bq_t = nc.dram_tensor("bq_t", [MQ, 128], F32, kind="ExternalInput")
        bk_t = nc.dram_tensor("bk_t", [MK, 128], F32, kind="ExternalInput")
        bv_t = nc.dram_tensor("bv_t", [2, 512], BF16, kind="ExternalInput")
        bo_t = nc.dram_tensor("bo_t", [NE, 512], BF16, kind="ExternalInput")
    out = nc.dram_tensor("out", [TOK, E], F32, kind="ExternalOutput")

    with tile.TileContext(nc) as tc:
        with tc.tile_pool(name="const", bufs=1) as cpool:
            cos_sb = cpool.tile([128, TOK], BF16, tag="cos")
            nc.sync.dma_start(cos_sb[:], cos_t[:, :])
            sin_sb = cpool.tile([128, TOK], BF16, tag="sin")
            nc.sync.dma_start(sin_sb[:], sin_t[:, :])
            ones_col = cpool.tile([128, 1], BF16, tag="ones_col")
            nc.vector.memset(ones_col[:], 1.0)
            ones_row = cpool.tile([1, 128], BF16, tag="ones_row")
            nc.vector.memset(ones_row[:], 1.0)
            if use_mask:
                mb_sb = cpool.tile([128, NBLK], F32, tag="mb")
                nc.sync.dma_start(mb_sb[:], mb_t.rearrange("b p -> p b"))
            if use_bias:
                bq_sb = cpool.tile([128, MQ], F32, tag="bq")
                nc.sync.dma_start(bq_sb[:], bq_t.rearrange("m p -> p m"))
                bk_sb = cpool.tile([128, MK], F32, tag="bk")
                nc.sync.dma_start(bk_sb[:], bk_t.rearrange("m p -> p m"))
                bv_sb = cpool.tile([1, 2, 512], BF16, tag="bv")
                nc.sync.dma_start(bv_sb[:], bv_t[None, :, :])
                bo_sb = cpool.tile([1, NE, 512], BF16, tag="bo")
                nc.sync.dma_start(bo_sb[:], bo_t[None, :, :])

            with tc.tile_pool(name="persist", bufs=1) as ppool:
                qT = ppool.tile([128, MQ, TOK], BF16, tag="qT")
                kT = ppool.tile([128, MK, TOK], BF16, tag="kT")
                v_sb = ppool.tile([128, NBLK, HKV * D], BF16, tag="v")

                with (
                    tc.tile_pool(name="xt", bufs=1) as xpool,
                    tc.tile_pool(name="wstream", bufs=2) as wpool,
                    tc.tile_pool(name="rope", bufs=2) as rpool,
                    tc.tile_pool(name="attn", bufs=3) as apool,
                    tc.tile_pool(name="lr", bufs=2) as lrpool,
                    tc.tile_pool(name="bcb", bufs=2) as bpool,
                    tc.tile_pool(name="psA", bufs=4, space="PSUM") as psA,
                    tc.tile_pool(name="ps_s", bufs=1, space="PSUM") as ps_s,
                    tc.tile_pool(name="ps_l", bufs=1, space="PSUM") as ps_l,
                    tc.tile_pool(name="ps_bc", bufs=1, space="PSUM") as ps_bc,
                    tc.tile_pool(name="ps_pv", bufs=1, space="PSUM") as ps_pv,
                ):
                    xt = xpool.tile([128, KO, TOK], BF16, tag="xt")

                    # ---- V projection (psum rows = tokens); the first
                    # (half, n) group interleaves with the xT load so the
                    # PE ramps with the DMA stream instead of after it.
                    def v_group(half, n, xt_dma):
                        pss = [psA.tile([128, 512], F32, tag="ps",
                                        name=f"psv{half}{n}_{i}")
                               for i in range(4)]
                        for kc in range(4):
                            if xt_dma:
                                for ko in range(kc * 8, kc * 8 + 8):
                                    nc.sync.dma_start(xt[:, ko, :], xT[ko])
                            wvs = wpool.tile([128, KO * 128], BF16, tag="w")
                            nc.sync.dma_start(
                                wvs[:, 0:8 * 512],
                                wv_t[n, :, kc * 8:(kc + 1) * 8, :]
                                .rearrange("p a c -> p (a c)"))
                            for k8 in range(8):
                                ko = kc * 8 + k8
                                for mt in range(4):
                                    c0 = half * 512 + mt * 128
                                    nc.tensor.matmul(
                                        pss[mt][:],
                                        xt[:, ko, c0:c0 + 128],
                                        wvs[:, k8 * 512:(k8 + 1) * 512],
                                        start=(ko == 0),
                                        stop=(ko == KO - 1 and not use_bias))
                        if use_bias:
                            for mt in range(4):
                                nc.tensor.matmul(pss[mt][:], ones_row[:],
                                                 bv_sb[:, n, :],
                                                 start=False, stop=True)
                        for mt in range(4):
                            nc.scalar.copy(
                                v_sb[:, half * 4 + mt, n * 512:(n + 1) * 512],
                                pss[mt][:])

                    v_group(0, 0, xt_dma=True)
                    v_group(0, 1, xt_dma=False)
                    v_group(1, 0, xt_dma=False)
                    v_group(1, 1, xt_dma=False)

                    # ---- fused RoPE psum eviction for q/k ----
                    def rope_evict(ps, dst_ap, half, bias_sb, m):
                        qa = rpool.tile([128, 512], BF16, tag="qa")
                        if bias_sb is not None:
                            nc.scalar.add(qa[:], ps[:], bias_sb[:, m:m + 1])
                        else:
                            nc.scalar.copy(qa[:], ps[:])
                        qsh = rpool.tile([128, 512], BF16, tag="qsh")
                        nc.sync.dma_start(qsh[0:64, :], qa[64:128, :])
                        nc.sync.dma_start(qsh[64:128, :], qa[0:64, :])
                        ts = slice(half * 512, (half + 1) * 512)
                        nc.vector.tensor_mul(qa[:], qa[:], cos_sb[:, ts])
                        nc.vector.tensor_mul(qsh[:], qsh[:], sin_sb[:, ts])
                        nc.vector.tensor_add(dst_ap, qa[:], qsh[:])

                    def qk_mtile(w_dram, dst, bias_sb, m, half):
                        if half == 0:
                            wsb = wpool.tile([128, KO * 128], BF16, tag="w",
                                             name="wsb")
                            qk_mtile.wsb = wsb
                            nc.sync.dma_start(
                                wsb[:], w_dram[m].rearrange("p a c -> p (a c)"))
                        wsb = qk_mtile.wsb
                        ps = psA.tile([128, 512], F32, tag="ps")
                        for ko in range(KO):
                            nc.tensor.matmul(
                                ps[:], wsb[:, ko * 128:(ko + 1) * 128],
                                xt[:, ko, half * 512:(half + 1) * 512],
                                start=(ko == 0), stop=(ko == KO - 1))
                        rope_evict(ps, dst[:, m, half * 512:(half + 1) * 512],
                                   half, bias_sb, m)

                    # ---- K projection ----
                    for m in range(MK):
                        for half in range(2):
                            qk_mtile(wk_t, kT, bk_sb if use_bias else None,
                                     m, half)

                    # ---- attention, 3-stage pipeline over 64 slots ----
                    # iter j = (g=j//8 kv head, blk=j%8). Slot k emits:
                    #   stage A (iter k-8):  s matmul, exp
                    #   stage B (iter k-9):  l matmul, ln, rc=exp(-lr)
                    #   stage C (iter k-10): bc matmul, bcb copy, normalize
                    #   then pv matmul (iter k-9), after stage C's read of
                    #   the single pv psum buffer.
                    # Output overwrites qT[:, heads of g, blk] in place.
                    st = {}

                    def attn_slot(k):
                        ja = k - 8
                        if 0 <= ja < 64:
                            g, blk = ja // 8, ja % 8
                            tq = slice(blk * 128, (blk + 1) * 128)
                            s_ps = ps_s.tile([128, 512], F32, tag="s")
                            nc.tensor.matmul(
                                s_ps[:], kT[:, g, tq],
                                qT[:, g * G:(g + 1) * G, tq],
                                start=True, stop=True)
                            wT = apool.tile([128, 512], BF16, tag="wT")
                            if use_mask:
                                nc.scalar.activation(
                                    out=wT[:], in_=s_ps[:], func=AF.Exp,
                                    scale=SCALE, bias=mb_sb[:, blk:blk + 1])
                            else:
                                nc.scalar.activation(
                                    out=wT[:], in_=s_ps[:], func=AF.Exp,
                                    scale=SCALE)
                            st[ja] = {'wT': wT, 'g': g, 'tq': tq}
                        jb = k - 9
                        if 0 <= jb < 64:
                            it = st[jb]
                            l_ps = ps_l.tile([1, 512], F32, tag="l")
                            nc.tensor.matmul(l_ps[:], ones_col[:],
                                             it['wT'][:], start=True,
                                             stop=True)
                            lr = lrpool.tile([1, 512], F32, tag="lr")
                            nc.scalar.activation(out=lr[:], in_=l_ps[:],
                                                 func=AF.Ln)
                            rc = lrpool.tile([1, 512], BF16, tag="rc")
                            nc.scalar.activation(out=rc[:], in_=lr[:],
                                                 func=AF.Exp, scale=-1.0)
                            it['rc'] = rc
                        jc = k - 10
                        if 0 <= jc < 64:
                            it = st.pop(jc)
                            bc_ps = ps_bc.tile([128, 512], F32, tag="bc")
                            nc.tensor.matmul(bc_ps[:], ones_row[:],
                                             it['rc'][:], start=True,
                                             stop=True)
                            bcb = bpool.tile([128, 512], F32, tag="bcb")
                            nc.vector.tensor_copy(bcb[:], bc_ps[:])
                            g, tq = it['g'], it['tq']
                            nc.vector.tensor_mul(
                                qT[:, g * G:(g + 1) * G, tq],
                                it['pv'][:], bcb[:])
                        if 0 <= jb < 64:
                            it = st[jb]
                            pv_ps = ps_pv.tile([128, 512], F32, tag="pv")
                            nc.tensor.matmul(
                                pv_ps[:],
                                v_sb[:, it['tq'].start // 128,
                                     it['g'] * 128:(it['g'] + 1) * 128],
                                it['wT'][:], start=True, stop=True)
                            it['pv'] = pv_ps

                    # ---- Q projection with interleaved attention ----
                    slot = 0
                    for m in range(MQ):
                        for half in range(2):
                            qk_mtile(wq_t, qT, bq_sb if use_bias else None,
                                     m, half)
                            attn_slot(slot)
                            slot += 1
                    for k in range(slot, slot + 10):
                        attn_slot(k)

                # ============ Phase C: out projection ====================
                with (
                    tc.tile_pool(name="wo", bufs=4) as wopool,
                    tc.tile_pool(name="oe", bufs=4) as oepool,
                    tc.tile_pool(name="psC", bufs=8, space="PSUM") as psC,
                ):
                    for n in range(NE):
                        pss = [psC.tile([128, 512], F32, tag="ps",
                                        name=f"pso{n}_{i}")
                               for i in range(8)]
                        for kc in range(4):
                            wos = wopool.tile([128, 8, 512], BF16, tag="wo")
                            nc.sync.dma_start(
                                wos[:], wo_t[n, :, kc * 8:(kc + 1) * 8, :])
                            for grp in range(2):
                                for h8 in range(8):
                                    hk = kc * 8 + h8
                                    for mt in range(4):
                                        i = grp * 4 + mt
                                        t0 = i * 128
                                        nc.tensor.matmul(
                                            pss[i][:],
                                            qT[:, hk, t0:t0 + 128],
                                            wos[:, h8, :],
                                            start=(hk == 0),
                                            stop=(hk == MQ - 1
                                                  and not use_bias))
                        if use_bias:
                            for i in range(8):
                                nc.tensor.matmul(pss[i][:], ones_row[:],
                                                 bo_sb[:, n, :],
                                                 start=False, stop=True)
                        for i in range(8):
                            oe = oepool.tile([128, 512], F32, tag="oe")
                            nc.scalar.copy(oe[:], pss[i][:])
                            r0 = i * 128
                            nc.sync.dma_start(
                                out[r0:r0 + 128, n * 512:(n + 1) * 512],
                                oe[:])

    return _split_excess_waits(nc)


_NC_CACHE = {}


def _get_nc(flags):
    if flags not in _NC_CACHE:
        _NC_CACHE[flags] = _build(flags)
    return _NC_CACHE[flags]


def _prepare(x, wq, bq, wk, bk, wv, bv, wo, bo, mask):
    x = np.asarray(x, np.float32)
    wq = np.asarray(wq, np.float32)
    wk = np.asarray(wk, np.float32)
    wv = np.asarray(wv, np.float32)
    wo = np.asarray(wo, np.float32)
    bq = np.asarray(bq, np.float32)
    bk = np.asarray(bk, np.float32)
    bv = np.asarray(bv, np.float32)
    bo = np.asarray(bo, np.float32)
    mask = np.asarray(mask)

    use_bias = bool(bq.any() or bk.any() or bv.any() or bo.any())
    use_mask = not bool(mask.all())

    # weight layouts (shared across cores)
    wq_t = np.ascontiguousarray(
        wq.reshape(KO, 128, MQ, 128).transpose(2, 1, 0, 3)).astype(NPBF16)
    wk_t = np.ascontiguousarray(
        wk.reshape(KO, 128, MK, 128).transpose(2, 1, 0, 3)).astype(NPBF16)
    wv_t = np.ascontiguousarray(
        wv.reshape(KO, 128, 2, 512).transpose(2, 1, 0, 3)).astype(NPBF16)
    wo_t = np.ascontiguousarray(
        wo.reshape(MQ, 128, NE, 512).transpose(2, 1, 0, 3)).astype(NPBF16)

    # RoPE tables (positions are global sequence positions)
    inv = 1.0 / (ROPE_BASE ** (np.arange(0, D, 2, dtype=np.float32) / D))
    pos = np.arange(S, dtype=np.float32)
    ang = pos[:, None] * inv[None, :]                      # [S, 64]
    cos_full = np.concatenate([np.cos(ang), np.cos(ang)], -1).T  # [128, S]
    sin_half = np.sin(ang).T                               # [64, S]
    sin_eff = np.concatenate([-sin_half, sin_half], 0)     # [128, S]

    shards_per_b = NCORES // B                             # 4
    in_maps = []
    for c in range(NCORES):
        b = c // shards_per_b
        s0 = (c % shards_per_b) * TOK
        xs = x[b, s0:s0 + TOK]                             # [TOK, E]
        xTs = np.ascontiguousarray(xs.T).astype(NPBF16)    # [E, TOK]
        im = {
            "xT": xTs.reshape(KO, 128, TOK),
            "wq_t": wq_t, "wk_t": wk_t, "wv_t": wv_t, "wo_t": wo_t,
            "cos_t": np.ascontiguousarray(cos_full[:, s0:s0 + TOK]).astype(NPBF16),
            "sin_t": np.ascontiguousarray(sin_eff[:, s0:s0 + TOK]).astype(NPBF16),
        }
        if use_mask:
            mshard = mask[b, s0:s0 + TOK].reshape(NBLK, BS)
            im["mb_t"] = np.where(mshard, np.float32(0.0),
                                  np.float32(-80.0)).astype(np.float32)
        if use_bias:
            im["bq_t"] = bq.reshape(MQ, 128).copy()
            im["bk_t"] = bk.reshape(MK, 128).copy()
            im["bv_t"] = bv.reshape(2, 512).astype(NPBF16)
            im["bo_t"] = bo.reshape(NE, 512).astype(NPBF16)
        in_maps.append(im)

    return in_maps, (use_bias, use_mask)


def _assemble(results):
    shards_per_b = NCORES // B
    out = np.empty((B, S, E), np.float32)
    for c in range(NCORES):
        b = c // shards_per_b
        s0 = (c % shards_per_b) * TOK
        out[b, s0:s0 + TOK] = results[c]["out"]
    return out


def kernel(**inputs):
    in_maps, flags = _prepare(**inputs)
    nc = _get_nc(flags)
    res = run_bass_kernel_spmd(nc, in_maps, core_ids=list(range(NCORES)))
    return _assemble(res.results)
